# revision 22
# baseline (speedup 1.0000x reference)
"""MoE layer (top-2 of 8 experts) on 8 TRN2 NeuronCores.

Strategy (single device launch):
  Host: gate logits (tiny 8192x1024x8 sgemm), softmax + top-2 +
      renormalized weights, and the expert-parallel sharding decision.
  Device (one SPMD launch, 8 cores): each core runs 2 "slots"; a slot
      is (expert e, half of H) and processes all tokens routed to e:
      partial_y = relu(x @ W1[e][:, half] + b1) @ W2[e][half, :] * w_tok.
      Half-experts are assigned to slots sorted-balanced (big halves to
      one slot group, small to the other) so per-core work is ~sum n_e/8
      instead of max_e n_e. bf16 matmuls, fp32 PSUM.
  Host: sum the two H-halves and scatter-add the two scaled expert
      contributions per token (+ w-weighted b2 correction).

Startup-critical path: all device inputs are host-prearranged so every
DMA is one contiguous run per partition. The first slot's W1 arrives in
small chunk tiles ahead of everything else on the scalar ring; block 0's
x arrives in per-d-tile chunks, so the first matmul needs ~384KB of DMA.
x prefetches for blocks 1-2 are dep-paced behind block-0 activations so
they don't steal HBM bandwidth from the critical W1 stream. L1 of blocks
0+1 runs before L2 of block 0, covering the W2 stream with ~55us of
W1-only compute. The program drains on a 128-token block whose two
output DMAs go on different rings.
"""

import numpy as np
import ml_dtypes

import concourse.mybir as mybir
import concourse.tile as tile
from concourse import bacc
from concourse.bass_utils import run_bass_kernel_spmd

P = 128
N_CORES = 8
NS = 2          # slots (half-experts) per core
CB = 512        # token block
BF16 = mybir.dt.bfloat16
F32 = mybir.dt.float32
_bf16_np = ml_dtypes.bfloat16

_build_cache = {}


def _blocks(cap):
    """CB-sized token blocks with a trailing multiple-of-128 block."""
    out, pos = [], 0
    while cap - pos >= CB:
        out.append((pos, CB))
        pos += CB
    if cap - pos:
        out.append((pos, cap - pos))
    return out


def _w1_chunks(HQ):
    """First-processed slot's W1 h-column chunks (in units of columns)."""
    return [(0, P), (P, P)] + [
        (h0, 2 * P) for h0 in range(2 * P, min(8 * P, HQ), 2 * P)
    ] + [(h0, 4 * P) for h0 in range(8 * P, HQ, 4 * P)]


def _build_moe(D, HQ, O, caps):
    """Per-core program: NS slots, slot s = one (expert, H-half) over
    caps[s] padded routed tokens.

    Layer 1 keeps H on partitions (hT = W1-half.T-slices @ xT), layer 2
    puts tokens back on partitions (y = hT-slices.T @ W2-half). All
    inputs arrive host-prearranged in SBUF layout (contiguous DMA runs).
    """
    DO, HO, OO = D // P, HQ // P, O // 512
    CT = sum(caps)
    nc = bacc.Bacc(None, target_bir_lowering=False)
    xh = nc.dram_tensor("xh", [P, DO * CT], BF16, kind="ExternalInput")
    w1 = nc.dram_tensor("w1", [P, NS * DO * HQ], BF16, kind="ExternalInput")
    w2 = nc.dram_tensor("w2", [P, NS, HO, O], BF16, kind="ExternalInput")
    b1 = nc.dram_tensor("b1", [P, NS * HO], F32, kind="ExternalInput")
    wt = nc.dram_tensor("wt", [P, CT // P], F32, kind="ExternalInput")
    y = nc.dram_tensor("y", [CT, O], F32, kind="ExternalOutput")
    y_r = y.rearrange("(n p) o -> p n o", p=P)
    W1CH = _w1_chunks(HQ)
    with tile.TileContext(nc) as tc:
        with (
            tc.tile_pool(name="wp", bufs=1) as wp,
            tc.tile_pool(name="xp", bufs=2) as xp,
            tc.tile_pool(name="x0p", bufs=1) as x0p,
            tc.tile_pool(name="hp", bufs=2) as hp,
            tc.tile_pool(name="op", bufs=4) as op,
            tc.tile_pool(name="hps", bufs=3, space="PSUM") as hps,
            tc.tile_pool(name="yps", bufs=4, space="PSUM") as yps,
        ):
            # --- weight / const streams (scalar HWDGE ring, in order) ---
            w1f = []            # first slot W1 chunks: (h0, nh, flat tile)
            for k, (h0, nh) in enumerate(W1CH):
                t = wp.tile([P, DO * nh], BF16, tag=f"w1f{k}", name=f"w1f{k}")
                nc.scalar.dma_start(t[:], w1[:, DO * h0:DO * (h0 + nh)])
                w1f.append((h0, nh, t))
                if k == 3:
                    # b1 (128 tiny descriptors, ~1.3us of ring time) is
                    # needed at the first relu (~14us) — after chunk 3
                    b1_sb = wp.tile([P, NS * HO], F32, tag="b1", name="b1_sb")
                    nc.scalar.dma_start(b1_sb[:], b1[:])
                if k == len(W1CH) - 1:
                    # wt only at the first output scale (~65us)
                    wt_sb = wp.tile([P, CT // P], F32, tag="wt", name="wt_sb")
                    nc.scalar.dma_start(wt_sb[:], wt[:])
            w2f = []            # first slot W2 in two half tiles
            for k in range(2):
                t = wp.tile([P, HO // 2, O], BF16, tag=f"w2f{k}", name=f"w2f{k}")
                nc.scalar.dma_start(t[:], w2[:, 0, k * HO // 2:(k + 1) * HO // 2])
                w2f.append(t)
            w1r = [None]        # later slots: whole flat tiles
            w2r = [None]
            for s in range(1, NS):
                t1 = wp.tile([P, DO * HQ], BF16, tag=f"w1_{s}", name=f"w1_{s}")
                nc.scalar.dma_start(t1[:], w1[:, s * DO * HQ:(s + 1) * DO * HQ])
                t2 = wp.tile([P, HO, O], BF16, tag=f"w2_{s}", name=f"w2_{s}")
                nc.scalar.dma_start(t2[:], w2[:, s])
                w1r.append(t1)
                w2r.append(t2)

            def w1_slice(s, di, hi):
                if s > 0:
                    return w1r[s][:, di * HQ + hi * P:di * HQ + (hi + 1) * P]
                for (h0, nh, t) in w1f:
                    if h0 <= hi * P < h0 + nh:
                        loc = hi * P - h0
                        return t[:, di * nh + loc:di * nh + loc + P]
                raise AssertionError

            def w2_slice(s, hi, ot):
                if s > 0:
                    return w2r[s][:, hi, ot * 512:(ot + 1) * 512]
                return w2f[hi // (HO // 2)][:, hi % (HO // 2), ot * 512:(ot + 1) * 512]

            # --- main loop ---
            blks = []
            off = 0
            for s in range(NS):
                for (n0, cb) in _blocks(caps[s]):
                    blks.append((s, off + n0, cb))
                off += caps[s]
            hTs = {}
            acts = {}

            def do_l1(k):
                s, g0, cb = blks[k]
                xbase = DO * g0
                if k == 0:
                    # per-d-tile x chunks: first matmul waits on 1/8
                    xds = []
                    for di in range(DO):
                        xt = x0p.tile([P, CB], BF16, tag=f"x0d{di}",
                                      name=f"x0d{di}")[:, :cb]
                        nc.sync.dma_start(
                            xt[:], xh[:, xbase + di * cb:xbase + (di + 1) * cb]
                        )
                        xds.append(xt)
                    x_of = lambda di: xds[di]
                else:
                    x_sb = xp.tile([P, DO * CB], BF16, tag="x",
                                   name="x_sb")[:, :DO * cb]
                    dma = nc.sync.dma_start(x_sb[:], xh[:, xbase:xbase + DO * cb])
                    if k in (1, 2):
                        # pace early x prefetch behind block-0 L1 so it
                        # doesn't steal HBM from the critical W1 stream
                        hdep = min(2 if k == 1 else 12, HO - 1)
                        tile.add_dep_helper(
                            dma.ins, acts[(0, hdep)].ins,
                            reason="pace x prefetch behind W1 stream",
                        )
                    x_of = lambda di: x_sb[:, di * cb:(di + 1) * cb]
                hT = hp.tile([P, HO, CB], BF16, tag="h", name="hT")[:, :, :cb]
                hTs[k] = hT
                for hi in range(HO):
                    ph = hps.tile([P, CB], F32, tag="ph", name="ph")[:, :cb]
                    for di in range(DO):
                        nc.tensor.matmul(
                            ph[:],
                            w1_slice(s, di, hi),
                            x_of(di),
                            start=(di == 0),
                            stop=(di == DO - 1),
                        )
                    acts[(k, hi)] = nc.scalar.activation(
                        hT[:, hi], ph[:],
                        mybir.ActivationFunctionType.Relu,
                        bias=b1_sb[:, s * HO + hi:s * HO + hi + 1],
                    )

            def do_l2(k):
                s, g0, cb = blks[k]
                last = k == len(blks) - 1
                hT = hTs.pop(k)
                for ct in range(cb // P):
                    # hi outer / ot inner: both ot matmuls share the
                    # same stationary hT slice
                    yts = [yps.tile([P, 512], F32, tag="yp", name=f"yp{ot}")
                           for ot in range(OO)]
                    for hi in range(HO):
                        for ot in range(OO):
                            nc.tensor.matmul(
                                yts[ot][:],
                                hT[:, hi, ct * P:(ct + 1) * P],
                                w2_slice(s, hi, ot),
                                start=(hi == 0),
                                stop=(hi == HO - 1),
                            )
                    ncol = g0 // P + ct
                    for ot in range(OO):
                        o_sb = op.tile([P, 512], F32, tag="o")
                        nc.vector.tensor_scalar_mul(
                            o_sb[:], yts[ot][:], wt_sb[:, ncol:ncol + 1]
                        )
                        # split the drain of the very last block
                        eng = nc.scalar if (last and ot == 1) else nc.sync
                        eng.dma_start(
                            y_r[:, ncol, ot * 512:(ot + 1) * 512], o_sb[:]
                        )

            # L1 of blocks 0+1 run back-to-back so the W2 stream hides
            # behind ~55us of compute that needs only W1 + x
            if len(blks) >= 2:
                do_l1(0)
                do_l1(1)
                do_l2(0)
                do_l2(1)
                rest = range(2, len(blks))
            else:
                rest = range(len(blks))
            for k in rest:
                do_l1(k)
                do_l2(k)
    nc.finalize()
    return nc


def _pad128(n):
    return max(P, ((n + P - 1) // P) * P)


def kernel(x, W1, b1, W2, b2, gate_w, gate_b):
    x = np.ascontiguousarray(x, dtype=np.float32)
    W1 = np.asarray(W1, dtype=np.float32)
    b1 = np.asarray(b1, dtype=np.float32)
    W2 = np.asarray(W2, dtype=np.float32)
    b2 = np.asarray(b2, dtype=np.float32)
    gate_w = np.ascontiguousarray(gate_w, dtype=np.float32)
    gate_b = np.asarray(gate_b, dtype=np.float32)

    B, D = x.shape
    E, _, H = W1.shape
    O = W2.shape[2]
    DO = D // P
    HQ = H // NS
    HO = HQ // P
    assert E == N_CORES and D % P == 0 and H % (NS * P) == 0

    # ---- Host: gating + top-2 routing (the sharding decision) ----
    lg = x.astype(np.float64) @ gate_w.astype(np.float64) + gate_b
    lg -= lg.max(axis=1, keepdims=True)
    probs = np.exp(lg)
    probs /= probs.sum(axis=1, keepdims=True)
    order = np.argsort(-probs, axis=1, kind="stable")[:, :2]
    p_top = np.take_along_axis(probs, order, axis=1)
    w_top = (p_top / p_top.sum(axis=1, keepdims=True)).astype(np.float32)

    idx_e, wt_e = [], []
    for e in range(E):
        m0 = order[:, 0] == e
        m1 = order[:, 1] == e
        sel = m0 | m1
        idx = np.nonzero(sel)[0]
        w = np.where(m0[sel], w_top[sel, 0], w_top[sel, 1]).astype(np.float32)
        idx_e.append(idx)
        wt_e.append(w)

    # ---- Balanced slot assignment: NS half-experts per core ----
    units = sorted(
        [(len(idx_e[e]), e, q) for e in range(E) for q in range(NS)],
        key=lambda t: (-t[0], t[1], t[2]),
    )
    groups = [units[p * N_CORES:(p + 1) * N_CORES] for p in range(NS)]
    caps = [_pad128(max(u[0] for u in g)) for g in groups]
    # process tail-less slots first, then bigger tails, so the program
    # drains on the smallest trailing block
    proc = sorted(
        range(NS),
        key=lambda s: (caps[s] % CB != 0, -(caps[s] % CB), -caps[s]),
    )
    groups = [groups[s] for s in proc]
    caps = tuple(caps[s] for s in proc)
    CT = sum(caps)

    key = ("moe", D, HQ, O, caps)
    if key not in _build_cache:
        _build_cache[key] = _build_moe(D, HQ, O, caps)
    nc = _build_cache[key]

    # ---- Build per-core inputs (SBUF layouts, contiguous DMA runs) ----
    x_bf = x.astype(_bf16_np)
    # per expert: [DO, P, n_e] so block slices transpose cheaply
    xTe = {e: np.ascontiguousarray(x_bf[idx_e[e]].T.reshape(DO, P, -1))
           for e in range(E)}
    W1_bf = W1.astype(_bf16_np)
    W2_bf = W2.astype(_bf16_np)
    W1CH = _w1_chunks(HQ)
    in_maps = []
    for c in range(N_CORES):
        slots = [groups[p][c] for p in range(NS)]
        xhh = np.zeros((P, DO * CT), dtype=_bf16_np)
        w1h = np.empty((P, NS * DO * HQ), dtype=_bf16_np)
        w2h = np.empty((P, NS, HO, O), dtype=_bf16_np)
        b1h = np.zeros((P, NS * HO), dtype=np.float32)
        wth = np.zeros((P, CT // P), dtype=np.float32)
        off = 0
        for s, (n_u, e, q) in enumerate(slots):
            hsl = slice(q * HQ, (q + 1) * HQ)
            # x: block-major, per block [P, DO*cb] contiguous
            for (n0, cb) in _blocks(caps[s]):
                blk = np.zeros((P, DO, cb), dtype=_bf16_np)
                lo, hi_ = n0, min(n_u, n0 + cb)
                if hi_ > lo:
                    blk[:, :, :hi_ - lo] = \
                        xTe[e][:, :, lo:hi_].transpose(1, 0, 2)
                xbase = DO * (off + n0)
                xhh[:, xbase:xbase + DO * cb] = blk.reshape(P, DO * cb)
            # W1: slot 0 chunk-major [P, DO*nh] per chunk; else [P, DO*HQ]
            w1s = W1_bf[e][:, hsl]          # [D, HQ]
            base = s * DO * HQ
            if s == 0:
                for (h0, nh) in W1CH:
                    ch = w1s[:, h0:h0 + nh].reshape(DO, P, nh).transpose(1, 0, 2)
                    w1h[:, base + DO * h0:base + DO * (h0 + nh)] = \
                        ch.reshape(P, DO * nh)
            else:
                w1h[:, base:base + DO * HQ] = \
                    w1s.reshape(DO, P, HQ).transpose(1, 0, 2).reshape(P, -1)
            w2h[:, s] = W2_bf[e][hsl].reshape(HO, P, O).transpose(1, 0, 2)
            b1h[:, s * HO:(s + 1) * HO] = b1[e][hsl].reshape(HO, P).T
            wpad = np.zeros(caps[s], dtype=np.float32)
            wpad[:n_u] = wt_e[e]
            wth[:, off // P:(off + caps[s]) // P] = wpad.reshape(-1, P).T
            off += caps[s]
        in_maps.append({
            "xh": xhh,
            "w1": w1h,
            "w2": np.ascontiguousarray(w2h),
            "b1": b1h,
            "wt": wth,
        })

    res = run_bass_kernel_spmd(nc, in_maps, core_ids=list(range(N_CORES)))

    # ---- Host: combine H-halves / experts, add gated b2 ----
    out = np.zeros((B, O), dtype=np.float32)
    for c in range(N_CORES):
        yc = res.results[c]["y"]
        off = 0
        for s in range(NS):
            n_u, e, q = groups[s][c]
            if n_u:
                out[idx_e[e]] += yc[off:off + n_u]
            off += caps[s]
    if np.any(b2):
        out += w_top[:, 0, None] * b2[order[:, 0]]
        out += w_top[:, 1, None] * b2[order[:, 1]]
    return out
